# revision 1
# baseline (speedup 1.0000x reference)
"""Trainium2 Bass kernel for DecoupledMLPGaussianActor.

32 independent per-head MLPs (512 -> 1024 -> 1024 -> 1) over batch 4096,
then diagonal-Gaussian log_prob of `act` under N(mu, exp(log_std)).

Sharding: expert/head-parallel — 4 heads per NeuronCore x 8 cores.
Each core computes muT[4, 4096] for its heads; the tiny logp tail
(elementwise over [4096, 32] + sum) is host-side gather glue.

On-chip layout: features on partitions, batch on the free dim
("transposed activations"), so W1/W2 DRAM layouts feed matmul lhsT
directly and layer-N output partitions line up as layer-N+1's
contraction dim. Matmuls run in float32r (full PE rate for free dim
>= 256, ~tf32 accuracy; fp32 would be 4x slower).
"""

import sys

sys.path.insert(0, "/opt/trn_rl_repo")
sys.path.insert(0, "/root/.axon_site/_ro/trn_rl_repo")

import numpy as np

import concourse.bass as bass  # noqa: F401  (engine types referenced via nc)
import concourse.mybir as mybir
import concourse.tile as tile
from concourse import bacc
from concourse.bass_utils import run_bass_kernel_spmd

# Problem shapes (hardcoded per contract)
OBS_DIM, ACT_DIM = 512, 32
H1, H2 = 1024, 1024
BATCH = 4096
LOG2PI = float(np.log(2.0 * np.pi))

N_CORES = 8
HPC = ACT_DIM // N_CORES          # heads per core = 4
P = 128                           # partitions
NB = 512                          # batch tile (free dim per matmul)
NCHUNK = BATCH // NB              # 8
K1T = OBS_DIM // P                # 4  k-tiles layer1
M1T = H1 // P                     # 8  m-tiles layer1
K2T = H1 // P                     # 8  k-tiles layer2
M2T = H2 // P                     # 8  m-tiles layer2

F32 = mybir.dt.float32
F32R = mybir.dt.float32r


def build_bass(trace_scopes=False):
    nc = bacc.Bacc(trn_type="TRN2")

    obsT = nc.dram_tensor("obsT", [OBS_DIM, BATCH], F32R, kind="ExternalInput")
    w1 = nc.dram_tensor("w1", [HPC, OBS_DIM, H1], F32R, kind="ExternalInput")
    w2 = nc.dram_tensor("w2", [HPC, H1, H2], F32R, kind="ExternalInput")
    # host-prepped: w3r[a, p, mt] = W3[head a, mt*128 + p]
    w3 = nc.dram_tensor("w3", [HPC, P, M2T], F32R, kind="ExternalInput")
    # host-prepped: b1r[a, p, mt] = b1[head a, mt*128 + p]
    b1 = nc.dram_tensor("b1", [HPC, P, M1T], F32, kind="ExternalInput")
    b2 = nc.dram_tensor("b2", [HPC, P, M2T], F32, kind="ExternalInput")
    muT = nc.dram_tensor("muT", [HPC, BATCH], F32, kind="ExternalOutput")

    with tile.TileContext(nc) as tc:
        with (
            tc.tile_pool(name="wpool", bufs=2) as wpool,
            tc.tile_pool(name="obspool", bufs=2) as obspool,
            tc.tile_pool(name="x1pool", bufs=2) as x1pool,
            tc.tile_pool(name="x2pool", bufs=3) as x2pool,
            tc.tile_pool(name="mupool", bufs=4) as mupool,
            tc.tile_pool(name="ps1", bufs=3, space="PSUM") as ps1,
            tc.tile_pool(name="ps2", bufs=3, space="PSUM") as ps2,
            tc.tile_pool(name="ps3", bufs=2, space="PSUM") as ps3,
        ):
            for a in range(HPC):
                # --- per-head weights (double-buffered across heads)
                w1t = wpool.tile([P, K1T * H1], F32R, tag="w1")
                for kt in range(K1T):
                    nc.sync.dma_start(
                        w1t[:, kt * H1 : (kt + 1) * H1],
                        w1[a, kt * P : (kt + 1) * P, :],
                    )
                w2t = wpool.tile([P, K2T * H2], F32R, tag="w2")
                for kt in range(K2T):
                    nc.sync.dma_start(
                        w2t[:, kt * H2 : (kt + 1) * H2],
                        w2[a, kt * P : (kt + 1) * P, :],
                    )
                w3t = wpool.tile([P, M2T], F32R, tag="w3")
                nc.sync.dma_start(w3t[:], w3[a])
                b1t = wpool.tile([P, M1T], F32, tag="b1")
                nc.sync.dma_start(b1t[:], b1[a])
                b2t = wpool.tile([P, M2T], F32, tag="b2")
                nc.sync.dma_start(b2t[:], b2[a])

                for c in range(NCHUNK):
                    cs = slice(c * NB, (c + 1) * NB)
                    # --- obs chunk (re-streamed per head; cheap)
                    obst = []
                    for kt in range(K1T):
                        ot = obspool.tile([P, NB], F32R, tag=f"obs{kt}")
                        nc.sync.dma_start(
                            ot[:], obsT[kt * P : (kt + 1) * P, cs]
                        )
                        obst.append(ot)

                    # --- layer 1: x1T[m1] = tanh(W1[:,m1].T @ obsT + b1)
                    x1tiles = []
                    for m1 in range(M1T):
                        ps = ps1.tile([P, NB], F32, tag="ps1")
                        for kt in range(K1T):
                            nc.tensor.matmul(
                                ps[:],
                                w1t[:, kt * H1 + m1 * P : kt * H1 + (m1 + 1) * P],
                                obst[kt][:],
                                start=(kt == 0),
                                stop=(kt == K1T - 1),
                            )
                        x1t = x1pool.tile([P, NB], F32R, tag=f"x1_{m1}")
                        nc.scalar.activation(
                            x1t[:], ps[:],
                            mybir.ActivationFunctionType.Tanh,
                            bias=b1t[:, m1 : m1 + 1],
                        )
                        x1tiles.append(x1t)

                    # --- layer 2 + layer 3 accumulation
                    pmu = ps3.tile([1, NB], F32, tag="ps3")
                    for m2 in range(M2T):
                        ps_ = ps2.tile([P, NB], F32, tag="ps2")
                        for k2 in range(K2T):
                            nc.tensor.matmul(
                                ps_[:],
                                w2t[:, k2 * H2 + m2 * P : k2 * H2 + (m2 + 1) * P],
                                x1tiles[k2][:],
                                start=(k2 == 0),
                                stop=(k2 == K2T - 1),
                            )
                        x2t = x2pool.tile([P, NB], F32R, tag="x2")
                        nc.scalar.activation(
                            x2t[:], ps_[:],
                            mybir.ActivationFunctionType.Tanh,
                            bias=b2t[:, m2 : m2 + 1],
                        )
                        # layer 3: mu accumulates across m2 into one psum row
                        nc.tensor.matmul(
                            pmu[0:1, :],
                            w3t[:, m2 : m2 + 1],
                            x2t[:],
                            start=(m2 == 0),
                            stop=(m2 == M2T - 1),
                            skip_group_check=True,
                        )

                    mu_sb = mupool.tile([1, NB], F32, tag="mu")
                    nc.scalar.copy(mu_sb[:], pmu[0:1, :])
                    nc.sync.dma_start(muT[a : a + 1, cs], mu_sb[:])

    nc.finalize()
    return nc


_CACHED = {}


def _get_nc():
    if "nc" not in _CACHED:
        _CACHED["nc"] = build_bass()
    return _CACHED["nc"]


def kernel(obs, act, W1, b1, W2, b2, W3, b3, log_std, _results_out=None):
    obs = np.asarray(obs, dtype=np.float32)
    act = np.asarray(act, dtype=np.float32)
    W1 = np.asarray(W1, dtype=np.float32)
    b1 = np.asarray(b1, dtype=np.float32)
    W2 = np.asarray(W2, dtype=np.float32)
    b2 = np.asarray(b2, dtype=np.float32)
    W3 = np.asarray(W3, dtype=np.float32)
    b3 = np.asarray(b3, dtype=np.float32)
    log_std = np.asarray(log_std, dtype=np.float32)

    nc = _get_nc()

    obsT = np.ascontiguousarray(obs.T)  # [512, 4096]
    in_maps = []
    for c in range(N_CORES):
        H = slice(c * HPC, (c + 1) * HPC)
        in_maps.append(
            {
                "obsT": obsT,
                "w1": np.ascontiguousarray(W1[H]),
                "w2": np.ascontiguousarray(W2[H]),
                "w3": np.ascontiguousarray(
                    W3[H].reshape(HPC, M2T, P).transpose(0, 2, 1)
                ),
                "b1": np.ascontiguousarray(
                    b1[H].reshape(HPC, M1T, P).transpose(0, 2, 1)
                ),
                "b2": np.ascontiguousarray(
                    b2[H].reshape(HPC, M2T, P).transpose(0, 2, 1)
                ),
            }
        )

    results = run_bass_kernel_spmd(nc, in_maps, core_ids=list(range(N_CORES)))
    if _results_out is not None:
        _results_out.append(results)

    muT = np.concatenate([results.results[c]["muT"] for c in range(N_CORES)], axis=0)
    mu = muT.T + b3[None, :]  # [4096, 32]

    # logp tail (host-side gather glue; ~0.8 MFLOP)
    z = (act - mu) * np.exp(-log_std)[None, :]
    logp = np.sum(-0.5 * (z * z + LOG2PI) - log_std[None, :], axis=-1)

    return mu.astype(np.float32), logp.astype(np.float32)


# revision 10
# speedup vs baseline: 5618.1156x; 5618.1156x over previous
"""Trainium2 Bass kernel for DecoupledMLPGaussianActor.

32 independent per-head MLPs (512 -> 1024 -> 1024 -> 1) over batch 4096,
then diagonal-Gaussian log_prob of `act` under N(mu, exp(log_std)).

Sharding: expert/head-parallel — 4 heads per NeuronCore x 8 cores.
Each core computes muT[4, 4096] for its heads; the tiny logp tail
(elementwise over [4096, 32] + sum) is host-side gather glue.

On-chip layout: features on partitions, batch on the free dim
("transposed activations"), so W1/W2 DRAM layouts feed matmul lhsT
directly and layer-N output partitions line up as layer-N+1's
contraction dim. Matmuls run in float32r (full PE rate for free dim
>= 256, ~tf32 accuracy; fp32 would be 4x slower).
"""

import sys

sys.path.insert(0, "/opt/trn_rl_repo")
sys.path.insert(0, "/root/.axon_site/_ro/trn_rl_repo")

import numpy as np

import concourse.bass as bass  # noqa: F401  (engine types referenced via nc)
import concourse.mybir as mybir
import concourse.tile as tile
from concourse import bacc
from concourse.bass_utils import run_bass_kernel_spmd

# Problem shapes (hardcoded per contract)
OBS_DIM, ACT_DIM = 512, 32
H1, H2 = 1024, 1024
BATCH = 4096
LOG2PI = float(np.log(2.0 * np.pi))

N_CORES = 8
HPC = ACT_DIM // N_CORES          # heads per core = 4
P = 128                           # partitions
NB = 512                          # batch tile (free dim per matmul)
NCHUNK = BATCH // NB              # 8
K1T = OBS_DIM // P                # 4  k-tiles layer1
M1T = H1 // P                     # 8  m-tiles layer1
K2T = H1 // P                     # 8  k-tiles layer2
M2T = H2 // P                     # 8  m-tiles layer2

F32 = mybir.dt.float32
F32R = mybir.dt.float32r


def build_bass(trace_scopes=False):
    nc = bacc.Bacc(trn_type="TRN2")

    obsT = nc.dram_tensor("obsT", [OBS_DIM, BATCH], F32R, kind="ExternalInput")
    w1 = nc.dram_tensor("w1", [HPC, OBS_DIM, H1], F32R, kind="ExternalInput")
    w2 = nc.dram_tensor("w2", [HPC, H1, H2], F32R, kind="ExternalInput")
    # host-prepped: w3r[a, p, mt] = W3[head a, mt*128 + p]
    w3 = nc.dram_tensor("w3", [HPC, P, M2T], F32, kind="ExternalInput")
    # host-prepped: b1r[a, p, mt] = b1[head a, mt*128 + p]
    b1 = nc.dram_tensor("b1", [HPC, P, M1T], F32, kind="ExternalInput")
    b2 = nc.dram_tensor("b2", [HPC, P, M2T], F32, kind="ExternalInput")
    muT = nc.dram_tensor("muT", [HPC, BATCH], F32, kind="ExternalOutput")

    MULT = mybir.AluOpType.mult
    ADD = mybir.AluOpType.add

    with tile.TileContext(nc) as tc:
        with (
            tc.tile_pool(name="wpool", bufs=2) as wpool,
            tc.tile_pool(name="obspool", bufs=2) as obspool,
            tc.tile_pool(name="x1pool", bufs=2) as x1pool,
            tc.tile_pool(name="x2pool", bufs=3) as x2pool,
            tc.tile_pool(name="accpool", bufs=2) as accpool,
            tc.tile_pool(name="accbig", bufs=1) as accbig,
            tc.tile_pool(name="mupool", bufs=4) as mupool,
            tc.tile_pool(name="cpool", bufs=1) as cpool,
            tc.tile_pool(name="ps1", bufs=3, space="PSUM") as ps1,
            tc.tile_pool(name="ps2", bufs=3, space="PSUM") as ps2,
            tc.tile_pool(name="ps3", bufs=2, space="PSUM") as ps3,
        ):
            # ones vector (f32r) for the 128->1 partition-reduction matmul
            ones_f32 = cpool.tile([P, 1], F32, tag="ones32")
            nc.gpsimd.memset(ones_f32[:], 1.0)
            ones_r = cpool.tile([P, 1], F32R, tag="onesr")
            nc.vector.tensor_copy(ones_r[:], ones_f32[:])

            obs_prefetch = []
            for kt in range(K1T):
                ot = obspool.tile([P, NB], F32R, tag=f"obs{kt}")
                nc.sync.dma_start(ot[:], obsT[kt * P : (kt + 1) * P, 0:NB])
                obs_prefetch.append(ot)

            for a in range(HPC):
                # --- layer-1 weights + bias first (chunk-0 critical path);
                # halves so the first matmul's tile-dep covers less data
                w1k = []
                for kt in range(K1T):
                    t = wpool.tile([P, H1], F32R, tag=f"w1k{kt}")
                    nc.sync.dma_start(
                        t[:, 0 : H1 // 2], w1[a, kt * P : (kt + 1) * P, 0 : H1 // 2]
                    )
                    nc.sync.dma_start(
                        t[:, H1 // 2 : H1], w1[a, kt * P : (kt + 1) * P, H1 // 2 : H1]
                    )
                    w1k.append(t)
                b1t = wpool.tile([P, M1T], F32, tag="b1")
                nc.sync.dma_start(b1t[:], b1[a])

                w2k = [None] * K2T
                w3t = None
                b2t = None
                acc_head = accbig.tile([P, BATCH], F32R, tag="accb")

                for c in range(NCHUNK):
                    cs = slice(c * NB, (c + 1) * NB)
                    # --- obs chunk (re-streamed per head; cheap)
                    if a == 0 and c == 0:
                        obst = obs_prefetch
                    else:
                        obst = []
                        for kt in range(K1T):
                            ot = obspool.tile([P, NB], F32R, tag=f"obs{kt}")
                            nc.sync.dma_start(
                                ot[:], obsT[kt * P : (kt + 1) * P, cs]
                            )
                            obst.append(ot)

                    # --- layer 1: x1T[m1] = tanh(W1[:,m1].T @ obsT + b1)
                    x1tiles = []
                    for m1 in range(M1T):
                        ps = ps1.tile([P, NB], F32, tag="ps1")
                        for kt in range(K1T):
                            nc.tensor.matmul(
                                ps[:],
                                w1k[kt][:, m1 * P : (m1 + 1) * P],
                                obst[kt][:],
                                start=(kt == 0),
                                stop=(kt == K1T - 1),
                            )
                        x1t = x1pool.tile([P, NB], F32R, tag=f"x1_{m1}")
                        nc.scalar.activation(
                            x1t[:], ps[:],
                            mybir.ActivationFunctionType.Tanh,
                            bias=b1t[:, m1 : m1 + 1],
                        )
                        x1tiles.append(x1t)

                    if c == 0:
                        # layer-2 weights stream in behind chunk-0 layer 1
                        for kt in range(K2T):
                            t = wpool.tile([P, H2], F32R, tag=f"w2k{kt}")
                            nc.sync.dma_start(
                                t[:], w2[a, kt * P : (kt + 1) * P, :]
                            )
                            w2k[kt] = t
                        w3t = wpool.tile([P, M2T], F32, tag="w3")
                        nc.sync.dma_start(w3t[:], w3[a])
                        b2t = wpool.tile([P, M2T], F32, tag="b2")
                        nc.sync.dma_start(b2t[:], b2[a])

                    # --- layer 2 (PE) + layer-3 partial products (DVE)
                    acc = None
                    for m2 in range(M2T):
                        ps_ = ps2.tile([P, NB], F32, tag="ps2")
                        for k2 in range(K2T):
                            nc.tensor.matmul(
                                ps_[:],
                                w2k[k2][:, m2 * P : (m2 + 1) * P],
                                x1tiles[k2][:],
                                start=(k2 == 0),
                                stop=(k2 == K2T - 1),
                            )
                        x2t = x2pool.tile([P, NB], F32R, tag="x2")
                        nc.scalar.activation(
                            x2t[:], ps_[:],
                            mybir.ActivationFunctionType.Tanh,
                            bias=b2t[:, m2 : m2 + 1],
                        )
                        # acc += x2t * W3[m2-tile]  (fused on DVE)
                        w3col = w3t[:, m2 : m2 + 1]
                        if m2 == 0:
                            acc = accpool.tile([P, NB], F32, tag="acc0")
                            nc.vector.tensor_scalar_mul(acc[:], x2t[:], w3col)
                        else:
                            dst = (
                                acc_head[:, cs]
                                if m2 == M2T - 1
                                else accpool.tile(
                                    [P, NB], F32, tag=f"acc{m2 % 2}"
                                )
                            )
                            nc.vector.scalar_tensor_tensor(
                                dst[:], x2t[:], w3col, acc[:], MULT, ADD
                            )
                            acc = dst

                # --- head epilogue: partition-reduce each chunk's acc via
                # one [128,1]x[128,512] matmul; overlaps next head's layer 1
                for c in range(NCHUNK):
                    cs = slice(c * NB, (c + 1) * NB)
                    pmu = ps3.tile([1, NB], F32, tag="ps3")
                    nc.tensor.matmul(
                        pmu[0:1, :],
                        ones_r[:],
                        acc_head[:, cs],
                        start=True,
                        stop=True,
                    )
                    mu_sb = mupool.tile([1, NB], F32, tag="mu")
                    nc.scalar.copy(mu_sb[:], pmu[0:1, :])
                    nc.sync.dma_start(muT[a : a + 1, cs], mu_sb[:])

    nc.finalize()
    return nc


_CACHED = {}
TRACE = False  # set by test harness to capture NTFF profile


def _get_nc():
    if "nc" not in _CACHED:
        _CACHED["nc"] = build_bass()
    return _CACHED["nc"]


def kernel(obs, act, W1, b1, W2, b2, W3, b3, log_std, _results_out=None):
    obs = np.asarray(obs, dtype=np.float32)
    act = np.asarray(act, dtype=np.float32)
    W1 = np.asarray(W1, dtype=np.float32)
    b1 = np.asarray(b1, dtype=np.float32)
    W2 = np.asarray(W2, dtype=np.float32)
    b2 = np.asarray(b2, dtype=np.float32)
    W3 = np.asarray(W3, dtype=np.float32)
    b3 = np.asarray(b3, dtype=np.float32)
    log_std = np.asarray(log_std, dtype=np.float32)

    nc = _get_nc()

    obsT = np.ascontiguousarray(obs.T)  # [512, 4096]
    in_maps = []
    for c in range(N_CORES):
        H = slice(c * HPC, (c + 1) * HPC)
        in_maps.append(
            {
                "obsT": obsT,
                "w1": np.ascontiguousarray(W1[H]),
                "w2": np.ascontiguousarray(W2[H]),
                "w3": np.ascontiguousarray(
                    W3[H].reshape(HPC, M2T, P).transpose(0, 2, 1)
                ),
                "b1": np.ascontiguousarray(
                    b1[H].reshape(HPC, M1T, P).transpose(0, 2, 1)
                ),
                "b2": np.ascontiguousarray(
                    b2[H].reshape(HPC, M2T, P).transpose(0, 2, 1)
                ),
            }
        )

    results = run_bass_kernel_spmd(
        nc, in_maps, core_ids=list(range(N_CORES)), trace=TRACE
    )
    if _results_out is not None:
        _results_out.append(results)

    muT = np.concatenate([results.results[c]["muT"] for c in range(N_CORES)], axis=0)
    mu = muT.T + b3[None, :]  # [4096, 32]

    # logp tail (host-side gather glue; ~0.8 MFLOP)
    z = (act - mu) * np.exp(-log_std)[None, :]
    logp = np.sum(-0.5 * (z * z + LOG2PI) - log_std[None, :], axis=-1)

    return mu.astype(np.float32), logp.astype(np.float32)


# revision 14
# speedup vs baseline: 5637.6721x; 1.0035x over previous
"""Trainium2 Bass kernel for DecoupledMLPGaussianActor.

32 independent per-head MLPs (512 -> 1024 -> 1024 -> 1) over batch 4096,
then diagonal-Gaussian log_prob of `act` under N(mu, exp(log_std)).

Sharding: expert/head-parallel — 4 heads per NeuronCore x 8 cores.
Each core computes muT[4, 4096] for its heads; the tiny logp tail
(elementwise over [4096, 32] + sum) is host-side gather glue.

On-chip layout: features on partitions, batch on the free dim
("transposed activations"), so W1/W2 DRAM layouts feed matmul lhsT
directly and layer-N output partitions line up as layer-N+1's
contraction dim. Matmuls run in float32r (full PE rate for free dim
>= 256, ~tf32 accuracy; fp32 would be 4x slower).
"""

import sys

sys.path.insert(0, "/opt/trn_rl_repo")
sys.path.insert(0, "/root/.axon_site/_ro/trn_rl_repo")

import numpy as np

import concourse.bass as bass  # noqa: F401  (engine types referenced via nc)
import concourse.mybir as mybir
import concourse.tile as tile
from concourse import bacc
from concourse.bass_utils import run_bass_kernel_spmd

# Problem shapes (hardcoded per contract)
OBS_DIM, ACT_DIM = 512, 32
H1, H2 = 1024, 1024
BATCH = 4096
LOG2PI = float(np.log(2.0 * np.pi))

N_CORES = 8
HPC = ACT_DIM // N_CORES          # heads per core = 4
P = 128                           # partitions
NB = 512                          # batch tile (free dim per matmul)
NCHUNK = BATCH // NB              # 8
K1T = OBS_DIM // P                # 4  k-tiles layer1
M1T = H1 // P                     # 8  m-tiles layer1
K2T = H1 // P                     # 8  k-tiles layer2
M2T = H2 // P                     # 8  m-tiles layer2

F32 = mybir.dt.float32
F32R = mybir.dt.float32r


def build_bass(trace_scopes=False):
    nc = bacc.Bacc(trn_type="TRN2")

    obsT = nc.dram_tensor("obsT", [OBS_DIM, BATCH], F32R, kind="ExternalInput")
    w1 = nc.dram_tensor("w1", [HPC, OBS_DIM, H1], F32R, kind="ExternalInput")
    w2 = nc.dram_tensor("w2", [HPC, H1, H2], F32R, kind="ExternalInput")
    # host-prepped: w3r[a, p, mt] = W3[head a, mt*128 + p]
    w3 = nc.dram_tensor("w3", [HPC, P, M2T], F32, kind="ExternalInput")
    # host-prepped: b1r[a, p, mt] = b1[head a, mt*128 + p]
    b1 = nc.dram_tensor("b1", [HPC, P, M1T], F32, kind="ExternalInput")
    b2 = nc.dram_tensor("b2", [HPC, P, M2T], F32, kind="ExternalInput")
    muT = nc.dram_tensor("muT", [HPC, BATCH], F32, kind="ExternalOutput")

    MULT = mybir.AluOpType.mult
    ADD = mybir.AluOpType.add

    with tile.TileContext(nc) as tc:
        with (
            tc.tile_pool(name="wpool", bufs=2) as wpool,
            tc.tile_pool(name="obspool", bufs=2) as obspool,
            tc.tile_pool(name="x1pool", bufs=2) as x1pool,
            tc.tile_pool(name="x2pool", bufs=3) as x2pool,
            tc.tile_pool(name="accpool", bufs=2) as accpool,
            tc.tile_pool(name="accbig", bufs=1) as accbig,
            tc.tile_pool(name="mupool", bufs=4) as mupool,
            tc.tile_pool(name="cpool", bufs=1) as cpool,
            tc.tile_pool(name="ps1", bufs=3, space="PSUM") as ps1,
            tc.tile_pool(name="ps2", bufs=3, space="PSUM") as ps2,
            tc.tile_pool(name="ps3", bufs=2, space="PSUM") as ps3,
        ):
            # ones vector (f32r) for the 128->1 partition-reduction matmul
            ones_f32 = cpool.tile([P, 1], F32, tag="ones32")
            nc.gpsimd.memset(ones_f32[:], 1.0)
            ones_r = cpool.tile([P, 1], F32R, tag="onesr")
            nc.vector.tensor_copy(ones_r[:], ones_f32[:])

            obs_prefetch = []
            for kt in range(K1T):
                ot = obspool.tile([P, NB], F32R, tag=f"obs{kt}")
                nc.sync.dma_start(ot[:], obsT[kt * P : (kt + 1) * P, 0:NB])
                obs_prefetch.append(ot)

            # chunks whose layer-3 acc is complete but not yet reduced:
            # (head, chunk-slice, acc_head tile). Reductions are emitted in
            # the next chunk's layer-1 window so they never stall the PE.
            pending = []

            def flush_pending():
                for aa, pcs, acc_ref in pending:
                    pmu = ps3.tile([1, NB], F32, tag="ps3")
                    nc.tensor.matmul(
                        pmu[0:1, :],
                        ones_r[:],
                        acc_ref[:, pcs],
                        start=True,
                        stop=True,
                    )
                    mu_sb = mupool.tile([1, NB], F32, tag="mu")
                    nc.scalar.copy(mu_sb[:], pmu[0:1, :])
                    nc.sync.dma_start(muT[aa : aa + 1, pcs], mu_sb[:])
                pending.clear()

            for a in range(HPC):
                # --- layer-1 weights + bias first (chunk-0 critical path);
                # halves so the first matmul's tile-dep covers less data
                w1k = []
                for kt in range(K1T):
                    t = wpool.tile([P, H1], F32R, tag=f"w1k{kt}")
                    npc = 4 if (a == 0 and kt == 0) else 2
                    step = H1 // npc
                    for j in range(npc):
                        nc.sync.dma_start(
                            t[:, j * step : (j + 1) * step],
                            w1[a, kt * P : (kt + 1) * P, j * step : (j + 1) * step],
                        )
                    w1k.append(t)
                b1t = wpool.tile([P, M1T], F32, tag="b1")
                nc.sync.dma_start(b1t[:], b1[a])

                w2k = [None] * K2T
                w3t = None
                b2t = None
                acc_head = accbig.tile([P, BATCH], F32R, tag="accb")

                for c in range(NCHUNK):
                    cs = slice(c * NB, (c + 1) * NB)
                    # --- obs chunk (re-streamed per head; cheap)
                    if a == 0 and c == 0:
                        obst = obs_prefetch
                    else:
                        obst = []
                        for kt in range(K1T):
                            ot = obspool.tile([P, NB], F32R, tag=f"obs{kt}")
                            nc.sync.dma_start(
                                ot[:], obsT[kt * P : (kt + 1) * P, cs]
                            )
                            obst.append(ot)

                    # --- layer 1: x1T[m1] = tanh(W1[:,m1].T @ obsT + b1)
                    x1tiles = []
                    for m1 in range(M1T):
                        ps = ps1.tile([P, NB], F32, tag="ps1")
                        for kt in range(K1T):
                            nc.tensor.matmul(
                                ps[:],
                                w1k[kt][:, m1 * P : (m1 + 1) * P],
                                obst[kt][:],
                                start=(kt == 0),
                                stop=(kt == K1T - 1),
                            )
                        x1t = x1pool.tile([P, NB], F32R, tag=f"x1_{m1}")
                        nc.scalar.activation(
                            x1t[:], ps[:],
                            mybir.ActivationFunctionType.Tanh,
                            bias=b1t[:, m1 : m1 + 1],
                        )
                        x1tiles.append(x1t)

                    # reduce the previous chunk's layer-3 acc while this
                    # chunk's layer 2 is still being fed
                    flush_pending()

                    if c == 0:
                        # layer-2 weights stream in behind chunk-0 layer 1
                        for kt in range(K2T):
                            t = wpool.tile([P, H2], F32R, tag=f"w2k{kt}")
                            nc.sync.dma_start(
                                t[:], w2[a, kt * P : (kt + 1) * P, :]
                            )
                            w2k[kt] = t
                        w3t = wpool.tile([P, M2T], F32, tag="w3")
                        nc.sync.dma_start(w3t[:], w3[a])
                        b2t = wpool.tile([P, M2T], F32, tag="b2")
                        nc.sync.dma_start(b2t[:], b2[a])

                    # --- layer 2 (PE) + layer-3 partial products (DVE)
                    acc = None
                    for m2 in range(M2T):
                        ps_ = ps2.tile([P, NB], F32, tag="ps2")
                        for k2 in range(K2T):
                            nc.tensor.matmul(
                                ps_[:],
                                w2k[k2][:, m2 * P : (m2 + 1) * P],
                                x1tiles[k2][:],
                                start=(k2 == 0),
                                stop=(k2 == K2T - 1),
                            )
                        x2t = x2pool.tile([P, NB], F32R, tag="x2")
                        nc.scalar.activation(
                            x2t[:], ps_[:],
                            mybir.ActivationFunctionType.Tanh,
                            bias=b2t[:, m2 : m2 + 1],
                        )
                        # acc += x2t * W3[m2-tile]  (fused on DVE)
                        w3col = w3t[:, m2 : m2 + 1]
                        if m2 == 0:
                            acc = accpool.tile([P, NB], F32, tag="acc0")
                            nc.vector.tensor_scalar_mul(acc[:], x2t[:], w3col)
                        else:
                            dst = (
                                acc_head[:, cs]
                                if m2 == M2T - 1
                                else accpool.tile(
                                    [P, NB], F32, tag=f"acc{m2 % 2}"
                                )
                            )
                            nc.vector.scalar_tensor_tensor(
                                dst[:], x2t[:], w3col, acc[:], MULT, ADD
                            )
                            acc = dst

                    pending.append((a, cs, acc_head))

            # final chunk's reduction
            flush_pending()

    nc.finalize()
    return nc


_CACHED = {}
TRACE = False  # set by test harness to capture NTFF profile


def _get_nc():
    if "nc" not in _CACHED:
        _CACHED["nc"] = build_bass()
    return _CACHED["nc"]


def kernel(obs, act, W1, b1, W2, b2, W3, b3, log_std, _results_out=None):
    obs = np.asarray(obs, dtype=np.float32)
    act = np.asarray(act, dtype=np.float32)
    W1 = np.asarray(W1, dtype=np.float32)
    b1 = np.asarray(b1, dtype=np.float32)
    W2 = np.asarray(W2, dtype=np.float32)
    b2 = np.asarray(b2, dtype=np.float32)
    W3 = np.asarray(W3, dtype=np.float32)
    b3 = np.asarray(b3, dtype=np.float32)
    log_std = np.asarray(log_std, dtype=np.float32)

    nc = _get_nc()

    obsT = np.ascontiguousarray(obs.T)  # [512, 4096]
    in_maps = []
    for c in range(N_CORES):
        H = slice(c * HPC, (c + 1) * HPC)
        in_maps.append(
            {
                "obsT": obsT,
                "w1": np.ascontiguousarray(W1[H]),
                "w2": np.ascontiguousarray(W2[H]),
                "w3": np.ascontiguousarray(
                    W3[H].reshape(HPC, M2T, P).transpose(0, 2, 1)
                ),
                "b1": np.ascontiguousarray(
                    b1[H].reshape(HPC, M1T, P).transpose(0, 2, 1)
                ),
                "b2": np.ascontiguousarray(
                    b2[H].reshape(HPC, M2T, P).transpose(0, 2, 1)
                ),
            }
        )

    results = run_bass_kernel_spmd(
        nc, in_maps, core_ids=list(range(N_CORES)), trace=TRACE
    )
    if _results_out is not None:
        _results_out.append(results)

    muT = np.concatenate([results.results[c]["muT"] for c in range(N_CORES)], axis=0)
    mu = muT.T + b3[None, :]  # [4096, 32]

    # logp tail (host-side gather glue; ~0.8 MFLOP)
    z = (act - mu) * np.exp(-log_std)[None, :]
    logp = np.sum(-0.5 * (z * z + LOG2PI) - log_std[None, :], axis=-1)

    return mu.astype(np.float32), logp.astype(np.float32)
